# revision 1
# baseline (speedup 1.0000x reference)
"""ConvergedInhibition TRN2 kernel.

The reference computes, per pixel (n,h,w), an FFT deconvolution along the
channel axis: y = ifft(fft(x)/fft(k)).real. Since k is fixed, this is a
circular convolution with g = ifft(1/fft(k)): y[i] = sum_j g[(i-j) mod C] x[j]
— a dense CxC circulant matmul applied to every pixel. Viewing activations[n]
as a [C, H*W] matrix A_n, the problem is out_n = G @ A_n: a [512,512] x
[512,3136] matmul per image, data-parallel over 32 images across 8 cores.

Implementation choices (measured on HW):
- fp16 I/O: activations/weights are rounded to fp16 on the host and the
  output is stored as fp16 (upcast on host). This halves HBM traffic, which
  is the roofline here, and costs ~2^-11 relative rounding (~3.6e-4 total).
- The deconv kernel g is concentrated in a ~224-wide circular window around
  t=288 (the reference center-pads k, shifting the delta to position 224).
  Rotating output rows by S=288 (z[r] = y[(r+S) mod C]) aligns the support
  so that only 3 of 4 K-chunks of the contraction carry mass; the 4th is
  dropped (adds ~7e-5 error). The rotation is undone by a host-side gather.
- Matmuls run at full PE rate in fp16, contracting K=3x128 into fp32 PSUM.
"""

import numpy as np

import concourse.bass as bass  # noqa: F401  (registers bass types)
import concourse.mybir as mybir
import concourse.tile as tile
from concourse import bacc
from concourse.bass_utils import run_bass_kernel_spmd

N_CORES = 8
N, C, H, W = 32, 512, 56, 56
HW = H * W                      # 3136
IMGS = N // N_CORES             # 4 images per core
P = 128                         # partitions
NCHUNK = C // P                 # 4
PT = 392                        # pixel tile (free dim), 3136 = 8*392
NPT = HW // PT                  # 8
CB = 784                        # DMA column block, 3136 = 4*784
NCB = HW // CB                  # 4
ROT = 288                       # output-row rotation aligning g's support
KEPT_D = (0, 1, 2)              # kept (zc - jc) mod 4 chunk distances
IO_DT = mybir.dt.float16
IO_NP = np.float16

_CACHE = {}

RAW = True  # hand-rolled bacc kernel (V4); False = TileContext fallback (V3)


def _build_nc_raw():
    """Hand-rolled engine programs with explicit semaphores.

    Same dataflow as the Tile version, but without Tile's ~6us semaphore-init
    preamble and ~9us reset/barrier epilogue. Streams:
      Sync:   act loads (img, cb, jc) + half the stores, one HWDGE ring
      Scalar: gt loads + the other half of the stores, the other ring
      Tensor: 3-matmul PSUM groups per (img, cb, zc, p2) tile
      Vector: PSUM->fp16 casts into o_sb
    o_sb is per-(img, zc) (no reuse) so stores never gate casts; a_sb is
    double-buffered over images gated by s_mm; the 8 PSUM banks are a ring
    gated by s_cast.
    """
    nc = bacc.Bacc("TRN2", target_bir_lowering=False, debug=False,
                   num_devices=N_CORES)
    act = nc.dram_tensor("act", [IMGS, C, HW], IO_DT, kind="ExternalInput")
    gt = nc.dram_tensor("gt", [C, C], IO_DT, kind="ExternalInput")
    out = nc.dram_tensor("out", [IMGS, C, HW], IO_DT, kind="ExternalOutput")

    act_v = act.ap().rearrange("n (jc p) m -> n jc p m", p=P)
    gt_v = gt.ap().rearrange("(jc p) r -> jc p r", p=P)
    out_v = out.ap().rearrange("n (zc p) m -> n zc p m", p=P)

    NKEPT = len(KEPT_D)
    P2 = NPT // NCB                       # p-tiles per column block (2)
    TILES_PER_CB = NCHUNK * P2            # 8 psum tiles per (img, cb)
    TILES_PER_IMG = NCB * TILES_PER_CB    # 32

    def tidx(img, cb, zc, p2):
        return img * TILES_PER_IMG + cb * TILES_PER_CB + zc * P2 + p2

    def store_ring(cb, zc):
        return "sync" if (cb + zc) % 2 == 0 else "scalar"

    from contextlib import ExitStack
    with ExitStack() as ctx:
        a_sb = [ctx.enter_context(
            nc.sbuf_tensor(f"a_sb{h}", [P, NCHUNK * HW], IO_DT)).ap()
            for h in range(2)]
        gt_sb = ctx.enter_context(
            nc.sbuf_tensor("gt_sb", [P, NCHUNK * C], IO_DT)).ap()
        o_sb = [[ctx.enter_context(
            nc.sbuf_tensor(f"o_sb{i}_{z}", [P, HW], IO_DT)).ap()
            for z in range(NCHUNK)] for i in range(IMGS)]
        psum = [ctx.enter_context(
            nc.psum_tensor(f"ps{i}", [P, 512], mybir.dt.float32)).ap()
            for i in range(8)]

        s_gt = nc.alloc_semaphore("s_gt")
        s_ld = [[nc.alloc_semaphore(f"s_ld{h}_{cb}") for cb in range(NCB)]
                for h in range(2)]
        # gpsimd (SWDGE) loads need their own sems: a sem driven by a
        # software DMA can't also be updated by HWDGE
        s_ldg = [nc.alloc_semaphore(f"s_ldg_{cb}") for cb in range(NCB)]
        s_mm = nc.alloc_semaphore("s_mm")
        s_cast = nc.alloc_semaphore("s_cast")
        s_st = {"sync": nc.alloc_semaphore("s_st_sync"),
                "scalar": nc.alloc_semaphore("s_st_sca")}
        all_sems = ([s_gt, s_mm, s_cast, s_st["sync"], s_st["scalar"]]
                    + [s for row in s_ld for s in row] + s_ldg)

        # Stage 0: clear semaphores; the Block-exit barrier orders this
        # before any use in the main block (sems are NOT zeroed on alloc
        # and must not carry values across executions).
        with nc.Block("clears") as blk:

            @blk.sync
            def _(sync):
                for s in all_sems:
                    sync.sem_clear(s)

        with nc.Block("main") as blk:

            def emit_loads(sync, img, cb):
                if img >= 2:
                    sync.wait_ge(s_mm, TILES_PER_IMG * (img - 2)
                                 + TILES_PER_CB * (cb + 1))
                for jc in range(NCHUNK):
                    sync.dma_start(
                        a_sb[img % 2][
                            :, jc * HW + cb * CB: jc * HW + (cb + 1) * CB],
                        act_v[img, jc, :, cb * CB:(cb + 1) * CB],
                    ).then_inc(s_ld[img % 2][cb], 16)

            @blk.sync
            def _(sync):
                n_store = 0
                for img in range(min(2, IMGS)):
                    for cb in range(NCB):
                        emit_loads(sync, img, cb)
                for img in range(IMGS):
                    for cb in range(NCB):
                        for zc in range(NCHUNK):
                            if store_ring(cb, zc) != "sync":
                                continue
                            sync.wait_ge(s_cast,
                                         tidx(img, cb, zc, P2 - 1) + 1)
                            sync.dma_start(
                                out_v[img, zc, :, cb * CB:(cb + 1) * CB],
                                o_sb[img][zc][:, cb * CB:(cb + 1) * CB],
                            ).then_inc(s_st["sync"], 16)
                            n_store += 1
                        if img + 2 < IMGS:
                            emit_loads(sync, img + 2, cb)
                sync.wait_ge(s_st["sync"], 16 * n_store)

            @blk.scalar
            def _(scalar):
                for jc in range(NCHUNK):
                    scalar.dma_start(
                        gt_sb[:, jc * C:(jc + 1) * C], gt_v[jc],
                    ).then_inc(s_gt, 16)
                n_store = 0
                for img in range(IMGS):
                    for cb in range(NCB):
                        for zc in range(NCHUNK):
                            if store_ring(cb, zc) != "scalar":
                                continue
                            scalar.wait_ge(
                                s_cast, tidx(img, cb, zc, P2 - 1) + 1)
                            scalar.dma_start(
                                out_v[img, zc, :, cb * CB:(cb + 1) * CB],
                                o_sb[img][zc][:, cb * CB:(cb + 1) * CB],
                            ).then_inc(s_st["scalar"], 16)
                            n_store += 1
                scalar.wait_ge(s_st["scalar"], 16 * n_store)

            @blk.tensor
            def _(tensor):
                tensor.wait_ge(s_gt, 16 * NCHUNK)
                # HAM warmup while the first act loads land: ~12 matmuls on
                # gt data into bank 7 (overwritten by the first real group
                # before its first read; start=True resets accumulation)
                for _ in range(12):
                    tensor.matmul(psum[7][:, :PT], gt_sb[:, :P],
                                  gt_sb[:, :PT], start=True, stop=True)
                for img in range(IMGS):
                    for cb in range(NCB):
                        tensor.wait_ge(s_ld[img % 2][cb],
                                       64 * (img // 2 + 1))
                        for zc in range(NCHUNK):
                            for p2 in range(P2):
                                t = tidx(img, cb, zc, p2)
                                if t >= 8:
                                    tensor.wait_ge(s_cast, t - 7)
                                p = cb * P2 + p2
                                for i, d in enumerate(KEPT_D):
                                    jc = (zc - d) % NCHUNK
                                    mm = tensor.matmul(
                                        psum[t % 8][:, :PT],
                                        gt_sb[:, jc * C + zc * P:
                                              jc * C + (zc + 1) * P],
                                        a_sb[img % 2][
                                            :, jc * HW + p * PT:
                                            jc * HW + (p + 1) * PT],
                                        start=(i == 0), stop=(i == NKEPT - 1),
                                    )
                                mm.then_inc(s_mm)

            @blk.vector
            def _(vector):
                for img in range(IMGS):
                    for cb in range(NCB):
                        for zc in range(NCHUNK):
                            for p2 in range(P2):
                                t = tidx(img, cb, zc, p2)
                                vector.wait_ge(s_mm, t + 1)
                                p = cb * P2 + p2
                                vector.tensor_copy(
                                    o_sb[img][zc][:, p * PT:(p + 1) * PT],
                                    psum[t % 8][:, :PT],
                                ).then_inc(s_cast)

    nc.compile()
    return nc


def _build_nc():
    if RAW:
        return _build_nc_raw()
    return _build_nc_tile()


def _build_nc_tile():
    nc = bacc.Bacc("TRN2", target_bir_lowering=False, debug=False,
                   num_devices=N_CORES)
    act = nc.dram_tensor("act", [IMGS, C, HW], IO_DT, kind="ExternalInput")
    gt = nc.dram_tensor("gt", [C, C], IO_DT, kind="ExternalInput")
    out = nc.dram_tensor("out", [IMGS, C, HW], IO_DT, kind="ExternalOutput")

    with tile.TileContext(nc) as tc:
        with (
            tc.tile_pool(name="gtp", bufs=1) as gtp,
            tc.tile_pool(name="apool", bufs=3) as apool,
            tc.tile_pool(name="opool", bufs=2) as opool,
            tc.tile_pool(name="ps", bufs=8, space="PSUM") as psp,
        ):
            # gt_sb cols [jc*C + zc*P : ...] hold GTs[jc*P:(jc+1)*P, zc*P:...]:
            # the stationary operand for psum[zc] += blk.T @ x[jc].
            # gt loads go on the scalar ring so the first act loads aren't
            # queued behind them on sync.
            gt_sb = gtp.tile([P, NCHUNK * C], IO_DT)
            gt_v = gt.ap().rearrange("(jc p) r -> jc p r", p=P)
            for jc in range(NCHUNK):
                nc.scalar.dma_start(gt_sb[:, jc * C:(jc + 1) * C], gt_v[jc])

            act_v = act.ap().rearrange("n (jc p) m -> n jc p m", p=P)
            out_v = out.ap().rearrange("n (zc p) m -> n zc p m", p=P)

            for img in range(IMGS):
                a_sb = apool.tile([P, NCHUNK * HW], IO_DT)
                # column-block loads so matmuls start after the first block
                for cb in range(NCB):
                    for jc in range(NCHUNK):
                        nc.sync.dma_start(
                            a_sb[:, jc * HW + cb * CB: jc * HW + (cb + 1) * CB],
                            act_v[img, jc, :, cb * CB:(cb + 1) * CB])
                o_sbs = [opool.tile([P, HW], IO_DT, tag=f"o{zc}",
                                    name=f"o_sb{zc}")
                         for zc in range(NCHUNK)]
                # cb-outer: each 0.8MB column block is fully consumed (all
                # zc) before the next is needed, so the PE keeps pace with
                # the loads instead of stalling per-zc.
                for cb in range(NCB):
                    for zc in range(NCHUNK):
                        o_sb = o_sbs[zc]
                        for p2 in range(NPT // NCB):
                            p = cb * (NPT // NCB) + p2
                            ps = psp.tile([P, PT], mybir.dt.float32)
                            for i, d in enumerate(KEPT_D):
                                jc = (zc - d) % NCHUNK
                                nc.tensor.matmul(
                                    ps[:],
                                    gt_sb[:, jc * C + zc * P: jc * C + (zc + 1) * P],
                                    a_sb[:, jc * HW + p * PT: jc * HW + (p + 1) * PT],
                                    start=(i == 0), stop=(i == len(KEPT_D) - 1),
                                )
                            nc.vector.tensor_copy(
                                o_sb[:, p * PT:(p + 1) * PT], ps[:])
                        # store each finished column block immediately,
                        # alternating DMA rings to spread the drain
                        eng = nc.scalar if (cb + zc) % 2 else nc.sync
                        eng.dma_start(
                            out_v[img, zc, :, cb * CB:(cb + 1) * CB],
                            o_sb[:, cb * CB:(cb + 1) * CB])
    nc.compile()
    return nc


def _make_gt(inhib_kernel: np.ndarray) -> np.ndarray:
    k = np.asarray(inhib_kernel, dtype=np.float64)
    g = np.real(np.fft.ifft(1.0 / np.fft.fft(k)))
    gs = np.roll(g, -ROT)  # gs[t'] = g[(t'+ROT) mod C]
    idx = (np.arange(C)[None, :] - np.arange(C)[:, None]) % C
    return np.ascontiguousarray(gs[idx].astype(IO_NP))  # GTs[j, r]


def kernel(activations, inhib_kernel):
    acts = np.asarray(activations, dtype=np.float32)
    assert acts.shape == (N, C, H, W), acts.shape
    gt_np = _make_gt(np.asarray(inhib_kernel))

    if "nc" not in _CACHE:
        _CACHE["nc"] = _build_nc()
    nc = _CACHE["nc"]

    acts_h = acts.reshape(N, C, HW).astype(IO_NP)
    in_maps = [
        {"act": np.ascontiguousarray(acts_h[c * IMGS:(c + 1) * IMGS]),
         "gt": gt_np}
        for c in range(N_CORES)
    ]
    res = run_bass_kernel_spmd(nc, in_maps, core_ids=list(range(N_CORES)))
    z = np.concatenate([r["out"] for r in res.results], axis=0)
    # un-rotate: y[i] = z[(i - ROT) mod C], upcast to fp32
    y = z[:, (np.arange(C) - ROT) % C, :].astype(np.float32)
    return y.reshape(N, C, H, W)



# revision 3
# speedup vs baseline: 1.7368x; 1.7368x over previous
"""ConvergedInhibition TRN2 kernel (fp8 correction-matmul version).

The reference computes, per pixel (n,h,w), an FFT deconvolution along the
channel axis: y = ifft(fft(x)/fft(k)).real. Since k is fixed, this is a
circular convolution with g = ifft(1/fft(k)): a dense CxC circulant matmul
applied to every pixel, data-parallel over 32 images across 8 cores.

This version exploits the structure y = x + c where c = (G - I) x is a small
correction (||c|| ~ 0.14 ||y||): the device computes only the correction from
fp8(e4m3)-quantized activations and stores it as fp8, halving HBM traffic in
both directions (the DMA roofline). The exact fp32 identity term is added
back on the host during unsharding, so quantization noise only enters scaled
by the correction magnitude (measured total rel err ~8e-3 vs 2e-2 budget).

Rotated frame: z[r] = y[(r+ROT) mod C] aligns the deconv impulse response h
(one-sided, support ~[0,224)) to the diagonal. Keeping chunk distances
d=(zc-jc) mod 4 in {0,1} covers t in [0, 128+q] per output row q (trunc err
~2e-3). For zc>=1 the two kept input chunks are adjacent in SBUF, so each
output tile is ONE fp8 DoubleRow matmul (K=256 at 2x PE rate, 392cyc). zc=0
wraps (jc=3,0) and uses two plain fp8 matmuls instead.

Engine layout (per core): gpsimd issues the 16 act loads on the SWDGE ring;
sync issues the 32 output stores; scalar loads gt then alternates with vector
on 784-col PSUM->fp8 pair-drains; tensor runs 160 matmuls (LDWEIGHTS
overlaps matmuls via the PE reorder window, measured 166ns/tile).
"""

import numpy as np
import ml_dtypes

import concourse.bass as bass  # noqa: F401  (registers bass types)
import concourse.mybir as mybir
from concourse import bacc
from concourse.bass_utils import run_bass_kernel_spmd

N_CORES = 8
N, C, H, W = 32, 512, 56, 56
HW = H * W                      # 3136
IMGS = N // N_CORES             # 4 images per core
P = 128                         # partitions
NCHUNK = C // P                 # 4
PT = 392                        # pixel tile (free dim), 3136 = 8*392
NPT = HW // PT                  # 8
ROT = 288                       # rotation aligning h's one-sided support
IO_DT = mybir.dt.float8e4
IO_NP = ml_dtypes.float8_e4m3   # matches TRN FP8_EXP4 semantics
N_WARMUP = 14                   # HAM clock-gate warmup matmuls

_CACHE = {}


def _build_nc():
    """Raw bacc engine programs with explicit semaphores."""
    nc = bacc.Bacc("TRN2", target_bir_lowering=False, debug=False,
                   num_devices=N_CORES)
    act = nc.dram_tensor("act", [IMGS, C, HW], IO_DT, kind="ExternalInput")
    gt = nc.dram_tensor("gt", [C, C], IO_DT, kind="ExternalInput")
    out = nc.dram_tensor("out", [IMGS, C, HW], IO_DT, kind="ExternalOutput")

    act_v = act.ap().rearrange("n (jc p) m -> n jc p m", p=P)
    gt_v = gt.ap().rearrange("(jc p) r -> jc p r", p=P)
    out_v = out.ap().rearrange("n (zc p) m -> n zc p m", p=P)

    ZCS = (1, 2, 3, 0)            # zc processing order (ascending chunk pairs)
    # loads (jc order 0..3) needed before each zc group can run
    LOADS_FOR_ZC = {1: 2, 2: 3, 3: 4, 0: 4}
    HALF = 2 * PT                 # store/drain granularity: 784 cols

    from contextlib import ExitStack
    with ExitStack() as ctx:
        a_sb = [ctx.enter_context(
            nc.sbuf_tensor(f"a_sb{i}", [P, NCHUNK * HW], IO_DT)).ap()
            for i in range(IMGS)]
        gt_sb = ctx.enter_context(
            nc.sbuf_tensor("gt_sb", [P, NCHUNK * C], IO_DT)).ap()
        o_sb = [[ctx.enter_context(
            nc.sbuf_tensor(f"o_sb{i}_{z}", [P, HW], IO_DT)).ap()
            for z in range(NCHUNK)] for i in range(IMGS)]
        # 4 x [P,1024] fp32 = 8 banks; tile t -> slot t%8 -> ps2[s//2], col (s%2)*512
        ps2 = [ctx.enter_context(
            nc.psum_tensor(f"ps{i}", [P, 1024], mybir.dt.float32)).ap()
            for i in range(4)]

        s_gt = nc.alloc_semaphore("s_gt")
        s_ld = [nc.alloc_semaphore(f"s_ld{i}") for i in range(IMGS)]
        s_mm = nc.alloc_semaphore("s_mm")
        s_cv = nc.alloc_semaphore("s_cv")    # vector pair-drains done
        s_cs = nc.alloc_semaphore("s_cs")    # scalar pair-drains done
        s_st = nc.alloc_semaphore("s_st")
        all_sems = [s_gt, s_mm, s_cv, s_cs, s_st] + s_ld

        a3 = [a.rearrange("p (jc m) -> p jc m", jc=NCHUNK) for a in a_sb]
        gt3 = gt_sb.rearrange("p (jc r) -> p jc r", jc=NCHUNK)
        psd = [p_.rearrange("p (two f) -> p two f", two=2) for p_ in ps2]

        def tile_idx(img, zci, pt):
            return (img * NCHUNK + zci) * NPT + pt

        def pair_engine(pair):       # alternate drains between the engines
            return "v" if pair % 2 == 0 else "s"

        # per-engine running pair counts, for store/psum-reuse waits
        v_done_at = {}
        s_done_at = {}
        nv = ns = 0
        for pr in range(IMGS * NCHUNK * NPT // 2):
            if pair_engine(pr) == "v":
                nv += 1
            else:
                ns += 1
            v_done_at[pr] = nv
            s_done_at[pr] = ns
        NV, NS = nv, ns

        with nc.Block("clears") as blk:

            @blk.sync
            def _(sync):
                for s in all_sems:
                    sync.sem_clear(s)

        with nc.Block("main") as blk:

            @blk.gpsimd
            def _(g):
                # SWDGE ring: all activation loads, full [128, 3136] chunks
                for img in range(IMGS):
                    for jc in range(NCHUNK):
                        g.dma_start(a3[img][:, jc], act_v[img, jc]
                                    ).then_inc(s_ld[img], 16)

            @blk.scalar
            def _(sc):
                for jc in range(NCHUNK):
                    sc.dma_start(gt_sb[:, jc * C:(jc + 1) * C], gt_v[jc]
                                 ).then_inc(s_gt, 16)
                # scalar's share of the PSUM -> fp8 pair-drains
                for img in range(IMGS):
                    for zci, zc in enumerate(ZCS):
                        for lp in range(NPT // 2):
                            pr = (tile_idx(img, zci, 0) // 2) + lp
                            if pair_engine(pr) != "s":
                                continue
                            sc.wait_ge(s_mm, 2 * pr + 2)
                            sc.activation(
                                o_sb[img][zc][:, lp * HALF:(lp + 1) * HALF],
                                psd[pr % 4][:, :, :PT],
                                mybir.ActivationFunctionType.Copy,
                            ).then_inc(s_cs, 1)

            @blk.vector
            def _(v):
                for img in range(IMGS):
                    for zci, zc in enumerate(ZCS):
                        for lp in range(NPT // 2):
                            pr = (tile_idx(img, zci, 0) // 2) + lp
                            if pair_engine(pr) != "v":
                                continue
                            v.wait_ge(s_mm, 2 * pr + 2)
                            v.tensor_copy(
                                o_sb[img][zc][:, lp * HALF:(lp + 1) * HALF],
                                psd[pr % 4][:, :, :PT],
                            ).then_inc(s_cv, 1)

            @blk.tensor
            def _(t):
                t.wait_ge(s_gt, 16 * NCHUNK)
                # HAM warmup on gt data into slot 7 (reset by tile 7's start)
                for _i in range(N_WARMUP):
                    t.matmul(ps2[3][:, 512:512 + PT], gt3[:, 0:2, :P],
                             gt3[:, 0:2, :PT], start=True, stop=True,
                             perf_mode=mybir.MatmulPerfMode.DoubleRow,
                             skip_group_check=True)
                for img in range(IMGS):
                    for zci, zc in enumerate(ZCS):
                        t.wait_ge(s_ld[img], 16 * LOADS_FOR_ZC[zc])
                        for pt in range(NPT):
                            ti = tile_idx(img, zci, pt)
                            if ti >= 8:
                                q = (ti - 8) // 2
                                if pair_engine(q) == "v":
                                    t.wait_ge(s_cv, v_done_at[q])
                                else:
                                    t.wait_ge(s_cs, s_done_at[q])
                            s = ti % 8
                            po = ps2[s // 2][:, (s % 2) * 512:(s % 2) * 512 + PT]
                            msl = slice(pt * PT, (pt + 1) * PT)
                            if zc >= 1:
                                t.matmul(
                                    po, gt3[:, zc - 1:zc + 1, zc * P:(zc + 1) * P],
                                    a3[img][:, zc - 1:zc + 1, msl],
                                    start=True, stop=True,
                                    perf_mode=mybir.MatmulPerfMode.DoubleRow,
                                ).then_inc(s_mm, 1)
                            else:
                                t.matmul(po, gt3[:, 3, 0:P],
                                         a3[img][:, 3, msl],
                                         start=True, stop=False)
                                t.matmul(po, gt3[:, 0, 0:P],
                                         a3[img][:, 0, msl],
                                         start=False, stop=True,
                                         ).then_inc(s_mm, 1)

            @blk.sync
            def _(sync):
                n_store = 0
                for img in range(IMGS):
                    for zci, zc in enumerate(ZCS):
                        base_pr = tile_idx(img, zci, 0) // 2
                        for h in range(2):
                            # half h covers pairs base_pr+2h, base_pr+2h+1
                            for q in (base_pr + 2 * h, base_pr + 2 * h + 1):
                                if pair_engine(q) == "v":
                                    sync.wait_ge(s_cv, v_done_at[q])
                                else:
                                    sync.wait_ge(s_cs, s_done_at[q])
                            sync.dma_start(
                                out_v[img, zc, :, h * 2 * HALF:(h + 1) * 2 * HALF],
                                o_sb[img][zc][:, h * 2 * HALF:(h + 1) * 2 * HALF],
                            ).then_inc(s_st, 16)
                            n_store += 1
                sync.wait_ge(s_st, 16 * n_store)

    nc.compile()
    return nc


def _make_gt(inhib_kernel: np.ndarray) -> np.ndarray:
    """Masked rotated circulant of the deconv correction, as fp8 lhsT.

    GTs[j, r] = h[(r - j) mod C] - delta[r==j], where h = roll(g, -ROT) and
    g = ifft(1/fft(k)); entries with chunk distance (r//P - j//P) mod 4 > 1
    are dropped (never touched by the kept matmuls).
    """
    k = np.asarray(inhib_kernel, dtype=np.float64)
    g = np.real(np.fft.ifft(1.0 / np.fft.fft(k)))
    h = np.roll(g, -ROT)
    r = np.arange(C)
    t = (r[None, :] - r[:, None]) % C          # [j, r]
    gts = h[t] - np.eye(C)
    d = ((r[None, :] // P) - (r[:, None] // P)) % NCHUNK
    gts *= (d <= 1)
    return np.ascontiguousarray(gts.astype(IO_NP))


def _prep_in_maps(acts_f32: np.ndarray, gt_np: np.ndarray):
    """Quantize activations to fp8 and shard per core."""
    acts8 = acts_f32.reshape(N, C, HW).astype(IO_NP)
    return [
        {"act": np.ascontiguousarray(acts8[c * IMGS:(c + 1) * IMGS]),
         "gt": gt_np}
        for c in range(N_CORES)
    ], acts8


def kernel(activations, inhib_kernel):
    acts = np.asarray(activations, dtype=np.float32)
    assert acts.shape == (N, C, H, W), acts.shape
    gt_np = _make_gt(np.asarray(inhib_kernel))

    if "nc" not in _CACHE:
        _CACHE["nc"] = _build_nc()
    nc = _CACHE["nc"]

    in_maps, acts8 = _prep_in_maps(acts, gt_np)
    res = run_bass_kernel_spmd(nc, in_maps, core_ids=list(range(N_CORES)))
    c_out = np.concatenate([r["out"] for r in res.results], axis=0)
    # z = x + c in the rotated frame (exact fp32 identity), then un-rotate
    z = acts.reshape(N, C, HW) + c_out.astype(np.float32)
    y = z[:, (np.arange(C) - ROT) % C, :]
    return np.ascontiguousarray(y.reshape(N, C, H, W))
